# revision 34
# baseline (speedup 1.0000x reference)
"""DampedLinOSSLayer Trainium2 kernel (8 NeuronCores, batch-sharded).

Math: per SSM channel p, the complex diagonal recurrence
    x_t = lam_p * x_{t-1} + bu_t,   lam_p = r_p * exp(i*th_p)
is factored through the gauge x_t = exp(i*th_p*t) * y_t so the hardware
tensor_tensor_scan (DVE) runs with a REAL per-channel coefficient on the
re/im planes independently.

v2 layout decisions (from HW microbenchmarks):
  - All rotation tables, scan operands, and matmul operands are bf16:
    DVE tensor_tensor gets the 2x perf mode only when every operand is a
    2-byte dtype in SBUF; the scan runs at ~2.09 ns/col in bf16 vs ~3.9
    in f32; bf16 matmuls are 2x faster than f32r below peak p-state.
  - The scan coefficient must be fp32-accurate (a 2^-9 error in r is
    amplified by the ~1/(1-r) memory of the recurrence).  Split
    r = r_hi * r_lo with r_hi exactly representable in bf16 and
    |ln r_lo| <= 2^-9: the scan uses r_hi (exact in bf16), and the
    residual magnitudes r_lo^{-t} / r_lo^{+t} are folded into the
    pre/post rotation tables (bounded by e^4 over t<2048).
  - Rotation tables span the full L=2048 (no chunk phase folding), so B
    and C need only a single weight copy and the scan is one op per
    (half, comp) over the whole sequence: no carry chaining at all.
  - Act engine does every PSUM->SBUF copy (with the f32->bf16 downcast).
    GpSimd cannot touch PSUM and measured ~2.5 ns/col on bf16 adds, and
    running it concurrently with DVE regressed DVE throughput (SBUF port
    contention), so every rotation op stays on DVE.

Per core: 4 batches of the 32.
  - input tiles [l,h] -> PE transpose -> inT [h, l] (bf16)
  - B-proj:  bu[p_half, comp, t] = bw^T @ inT      (PE, bf16)
  - pre-rotation (packed complex mul)              (DVE bf16 2x)
  - scan y = r_hi*y + c along t, full L=2048       (DVE scan, bf16)
  - post-rotation -> x                             (DVE bf16 2x)
  - C-proj + D-residual -> out[t, h]               (PE, PSUM-accumulated)
"""

import numpy as np

BATCH, LENGTH, HIDDEN, P = 32, 2048, 128, 256
N_CORES = 8
BPC = BATCH // N_CORES          # batches per core
CH = 512                        # chunk size (PSUM tile granularity)
NCH = LENGTH // CH              # 4 chunks
NBLK = CH // 128                # 4 token-blocks of 128 per chunk

_COMPILED = {}


def _build_program():
    import concourse.bacc as bacc
    import concourse.mybir as mybir
    from concourse.ap import AP
    from concourse.tile import TileContext

    def dup2(a):
        """Insert a stride-0 'which' dim after the partition dim."""
        return AP(a.tensor, a.offset, [list(a.ap[0]), [0, 2],
                                       *[list(x) for x in a.ap[1:]]])

    f32 = mybir.dt.float32
    bf16 = mybir.dt.bfloat16

    nc = bacc.Bacc("TRN2", target_bir_lowering=False, debug=False,
                   num_devices=N_CORES)

    # ---- DRAM tensors (per-core) ----
    xin = nc.dram_tensor("xin", [BPC, LENGTH, HIDDEN], f32,
                         kind="ExternalInput").ap()
    # B weights [h, comp, half, p] / C weights [p, comp, half, h] (bf16)
    bw = nc.dram_tensor("bw", [HIDDEN, 2, 2, 128], bf16,
                        kind="ExternalInput").ap()
    cw = nc.dram_tensor("cw", [128, 2, 2, HIDDEN], bf16,
                        kind="ExternalInput").ap()
    # rotation tables [p, half, which, comp, t] over full L (bf16)
    tp = nc.dram_tensor("tp", [128, 2, 2, 2, LENGTH], bf16,
                        kind="ExternalInput").ap()
    tq = nc.dram_tensor("tq", [128, 2, 2, 2, LENGTH], bf16,
                        kind="ExternalInput").ap()
    # scan coefficient r_hi broadcast [p, half, t] (bf16-exact)
    rbc = nc.dram_tensor("rbc", [128, 2, LENGTH], bf16,
                         kind="ExternalInput").ap()
    dw = nc.dram_tensor("dw", [HIDDEN, HIDDEN], bf16,
                        kind="ExternalInput").ap()
    eye16 = nc.dram_tensor("eye16", [128, 128], bf16,
                           kind="ExternalInput").ap()
    out = nc.dram_tensor("out", [BPC, LENGTH, HIDDEN], f32,
                         kind="ExternalOutput").ap()

    with TileContext(nc) as tc:
        with (
            tc.tile_pool(name="const", bufs=1) as cpool,
            tc.tile_pool(name="inat", bufs=2) as inat_pool,
            tc.tile_pool(name="intp", bufs=2) as intr_pool,   # inT per batch
            tc.tile_pool(name="busb", bufs=2) as busb_pool,   # bu bf16 chunk
            tc.tile_pool(name="t12", bufs=2) as t12_pool,     # mul scratch
            tc.tile_pool(name="cbuf", bufs=2) as cbuf_pool,   # scan in (full L)
            tc.tile_pool(name="ybuf", bufs=1) as ybuf_pool,   # scan out (full L)
            tc.tile_pool(name="t34", bufs=2) as t34_pool,     # post scratch
            tc.tile_pool(name="otb", bufs=2) as otb_pool,     # outT staging
            tc.tile_pool(name="obuf", bufs=2) as obuf_pool,   # out staging
            tc.tile_pool(name="pst", bufs=2, space="PSUM") as pst,
            tc.tile_pool(name="psb", bufs=2, space="PSUM") as psb,
            tc.tile_pool(name="pso", bufs=1, space="PSUM") as pso,
            tc.tile_pool(name="psq", bufs=1, space="PSUM") as psq,
        ):
            # ---- constants to SBUF ----
            bw_t = cpool.tile([HIDDEN, 2, 2, 128], bf16, tag="bw")
            cw_t = cpool.tile([128, 2, 2, HIDDEN], bf16, tag="cw")
            tp_t = cpool.tile([128, 2, 2, 2, LENGTH], bf16, tag="tp")
            tq_t = cpool.tile([128, 2, 2, 2, LENGTH], bf16, tag="tq")
            rbc_t = cpool.tile([128, 2, LENGTH], bf16, tag="rbc")
            dw_t = cpool.tile([HIDDEN, HIDDEN], bf16, tag="dw")
            eye16_t = cpool.tile([128, 128], bf16, tag="eye16")
            # order matters: first consumers first.  tq/cw/dw/eye16 are
            # deferred into the batch-0 body so their DMAs don't contend
            # with batch-0's table reads on the SBUF ports.
            nc.sync.dma_start(eye16_t[:], eye16[:])
            nc.sync.dma_start(bw_t[:], bw[:])
            # batch-0 input next (see loop below), then tp chunk-sliced so
            # the first pre-rotation only waits on its own 1MB slice.
            dma_order = [
                lambda: [nc.sync.dma_start(tp_t[:, h, :, :, CH * J:CH * (J + 1)],
                                           tp[:, h, :, :, CH * J:CH * (J + 1)])
                         for J in range(NCH) for h in range(2)],
                lambda: nc.sync.dma_start(rbc_t[:], rbc[:]),
            ]

            inputs = {}

            def emit_input(b):
                # load + cast + transpose input: inT [h, l] bf16
                inT = intr_pool.tile([HIDDEN, LENGTH], bf16, tag="inT",
                                     name=f"inT{b}")
                nat = inat_pool.tile([128, LENGTH // 128, HIDDEN], f32,
                                     tag="nat")
                nat16 = inat_pool.tile([128, LENGTH // 128, HIDDEN], bf16,
                                       tag="nat16")
                nc.sync.dma_start(
                    nat[:], xin[b].rearrange("(n p) h -> p n h", p=128))
                if b == 0:
                    for fn in dma_order:
                        fn()
                nc.scalar.copy(nat16[:], nat[:])
                for q in range(NCH):
                    tpp = pst.tile([HIDDEN, 4, 128], bf16, tag="tpp")
                    for j in range(4):
                        nc.tensor.transpose(
                            tpp[:, j, :], nat16[:, 4 * q + j, :], eye16_t[:])
                    nc.scalar.copy(
                        inT[:, CH * q:CH * (q + 1)],
                        tpp[:].rearrange("h f t -> h (f t)"))
                inputs[b] = inT

            emit_input(0)
            for b in range(BPC):
                inT = inputs[b]
                # ---- B-proj + pre-rotation, chunk by chunk -> cc full L ----
                cc = cbuf_pool.tile([128, 2, 2, LENGTH], bf16, tag="cc",
                                    name=f"cc{b}")
                for J in range(NCH):
                    tsl = slice(CH * J, CH * (J + 1))
                    for half in range(2):
                        bu = psb.tile([128, 2, CH], f32, tag="bu")
                        for comp in range(2):
                            nc.tensor.matmul(
                                bu[:, comp, :],
                                bw_t[:, comp, half],
                                inT[:, tsl],
                                start=True, stop=True)
                        busb = busb_pool.tile([128, 2, CH], bf16, tag="busb")
                        nc.scalar.copy(busb[:], bu[:])
                        t12 = t12_pool.tile([128, 2, 2, CH], bf16, tag="t12")
                        # one mul op: busb broadcast (stride-0) over `which`
                        nc.vector.tensor_mul(
                            t12[:], dup2(busb[:]), tp_t[:, half, :, :, tsl])
                        # cc[half, comp] = t12[w, 0] + t12[w, 1] -> packed add
                        nc.vector.tensor_add(
                            cc[:, half, :, tsl],
                            t12[:, :, 0, :], t12[:, :, 1, :])

                if b == 0:
                    # deferred constant DMAs: queued behind batch-0 input
                    # work so they don't contend with batch-0 table reads
                    nc.sync.dma_start(tq_t[:], tq[:])
                    nc.sync.dma_start(cw_t[:], cw[:])
                    nc.sync.dma_start(dw_t[:], dw[:])

                # ---- scans: y = r_hi * y + c, full L per (half, comp) ----
                yy = ybuf_pool.tile([128, 2, 2, LENGTH], bf16, tag="yy",
                                    name=f"yy{b}")
                for half in range(2):
                    for comp in range(2):
                        nc.vector.tensor_tensor_scan(
                            yy[:, half, comp, :],
                            rbc_t[:, half],
                            cc[:, half, comp, :],
                            0.0,
                            op0=mybir.AluOpType.mult,
                            op1=mybir.AluOpType.add)

                # next batch's input phase overlaps this batch's scan/post
                if b + 1 < BPC:
                    emit_input(b + 1)

                # ---- post-rotation + C-proj + D-residual, per chunk ----
                osb = obuf_pool.tile([128, LENGTH // 128, HIDDEN], f32,
                                     tag="osb")
                for J in range(NCH):
                    tsl = slice(CH * J, CH * (J + 1))
                    t34s = []
                    for half in range(2):
                        t34 = t34_pool.tile([128, 2, 2, CH], bf16, tag="t34",
                                            name=f"t34_{b}_{J}_{half}")
                        nc.vector.tensor_mul(
                            t34[:], dup2(yy[:, half, :, tsl]),
                            tq_t[:, half, :, :, tsl])
                        t34s.append(t34)

                    # the post-rotation add (sum over y-comp c) is folded
                    # into the C-proj contraction: same cw lhsT for both c
                    outT = pso.tile([HIDDEN, CH], f32, tag="outT")
                    first = True
                    for comp in range(2):
                        for half in range(2):
                            for c in range(2):
                                nc.tensor.matmul(
                                    outT[:],
                                    cw_t[:, comp, half],
                                    t34s[half][:, comp, c, :],
                                    start=first, stop=False)
                                first = False
                    nc.tensor.matmul(
                        outT[:], dw_t[:], inT[:, tsl],
                        start=False, stop=True)
                    oT = otb_pool.tile([HIDDEN, CH], bf16, tag="oT")
                    nc.scalar.copy(oT[:], outT[:])
                    tpo = psq.tile([128, 4, HIDDEN], bf16, tag="tpo")
                    for i in range(NBLK):
                        nc.tensor.transpose(
                            tpo[:, i, :], oT[:, 128 * i:128 * (i + 1)],
                            eye16_t[:])
                    nc.scalar.copy(
                        osb[:, 4 * J:4 * (J + 1), :],
                        tpo[:].rearrange("t f h -> t (f h)").rearrange(
                            "t (f h) -> t f h", h=HIDDEN))
                nc.gpsimd.dma_start(
                    out[b].rearrange("(n p) h -> p n h", p=128), osb[:])

    nc.compile()
    return nc


def _host_constants(A_diag, G_diag, steps, B, C, D):
    """Parameter projection + eigenvalues + full-length rotation tables."""
    import ml_dtypes

    A = A_diag.astype(np.float64)
    G = G_diag.astype(np.float64)
    st = steps.astype(np.float64)
    step = 1.0 / (1.0 + np.exp(-st))
    g = np.maximum(G, 0.0)
    denom = np.maximum(step * step, 1e-6)
    s = step * g
    base = np.sqrt(np.maximum(1.0 + s, 1e-6))
    a_low = (2.0 + s - 2.0 * base) / denom
    a_high = (2.0 + s + 2.0 * base) / denom
    a = a_low + np.maximum(A - a_low, 0.0) - np.maximum(A - a_high, 0.0)
    S = 1.0 / (1.0 + step * g)
    T = S + 1.0 - step * step * S * a
    imag = np.sqrt(np.maximum(S - 0.25 * T * T, 0.0))
    lam = 0.5 * T + 1j * imag                      # [P] complex128
    r = np.abs(lam)
    th = np.angle(lam)

    # r = r_hi * r_lo with r_hi bf16-exact; fold r_lo^{+-t} into the tables
    r_hi = np.asarray(r.astype(np.float32), dtype=ml_dtypes.bfloat16)
    r_hi64 = r_hi.astype(np.float64)
    log_rlo = np.log(r) - np.log(r_hi64)           # |.| <= ~2^-9

    t = np.arange(LENGTH, dtype=np.float64)
    bf = ml_dtypes.bfloat16

    tp = np.zeros((128, 2, 2, 2, LENGTH), np.float32)
    tq = np.zeros((128, 2, 2, 2, LENGTH), np.float32)
    rbc = np.zeros((128, 2, LENGTH), np.float32)
    for half in range(2):
        psl = slice(128 * half, 128 * (half + 1))
        ang = th[psl, None] * t[None, :]
        cos_m = np.cos(ang)
        sin_m = np.sin(ang)
        mag_pre = np.exp(-log_rlo[psl, None] * t[None, :])   # r_lo^{-t}
        mag_post = np.exp(+log_rlo[psl, None] * t[None, :])  # r_lo^{+t}
        # pre: c = exp(-i th t) * r_lo^{-t} * bu
        tp[:, half, 0, 0, :] = cos_m * mag_pre
        tp[:, half, 0, 1, :] = sin_m * mag_pre
        tp[:, half, 1, 0, :] = -sin_m * mag_pre
        tp[:, half, 1, 1, :] = cos_m * mag_pre
        # post: x = exp(+i th t) * r_lo^{+t} * y
        tq[:, half, 0, 0, :] = cos_m * mag_post
        tq[:, half, 0, 1, :] = -sin_m * mag_post
        tq[:, half, 1, 0, :] = sin_m * mag_post
        tq[:, half, 1, 1, :] = cos_m * mag_post
        rbc[:, half, :] = r_hi64[psl, None]

    Br = B[..., 0].astype(np.float64)
    Bi = B[..., 1].astype(np.float64)
    Cr = C[..., 0].astype(np.float64)
    Ci = C[..., 1].astype(np.float64)
    bw = np.zeros((HIDDEN, 2, 2, 128), np.float32)
    cw = np.zeros((128, 2, 2, HIDDEN), np.float32)
    for half in range(2):
        psl = slice(128 * half, 128 * (half + 1))
        bw[:, 0, half] = Br[psl].T                 # lhsT [h, p]
        bw[:, 1, half] = Bi[psl].T
        cw[:, 0, half] = Cr[:, psl].T              # out = Cr*xr - Ci*xi
        cw[:, 1, half] = -Ci[:, psl].T

    dwm = np.diag(D.astype(np.float64)).astype(np.float32)
    return dict(
        bw=bw.astype(bf), cw=cw.astype(bf),
        tp=tp.astype(bf), tq=tq.astype(bf),
        rbc=rbc.astype(bf), dw=dwm.astype(bf),
        eye16=np.eye(128, dtype=np.float32).astype(bf),
    )


def kernel(inputs, A_diag, G_diag, steps, B, C, D):
    from concourse import bass_utils

    inputs = np.asarray(inputs, np.float32)
    consts = _host_constants(np.asarray(A_diag), np.asarray(G_diag),
                             np.asarray(steps), np.asarray(B), np.asarray(C),
                             np.asarray(D))

    if "prog" not in _COMPILED:
        _COMPILED["prog"] = _build_program()
    nc = _COMPILED["prog"]

    in_maps = []
    for core in range(N_CORES):
        m = dict(consts)
        m["xin"] = np.ascontiguousarray(inputs[BPC * core: BPC * (core + 1)])
        in_maps.append(m)
    res = bass_utils.run_bass_kernel_spmd(nc, in_maps,
                                          core_ids=list(range(N_CORES)))
    out = np.concatenate([res.results[i]["out"] for i in range(N_CORES)],
                         axis=0)
    return out.astype(np.float32)


# revision 36
# speedup vs baseline: 1.1200x; 1.1200x over previous
"""DampedLinOSSLayer Trainium2 kernel (8 NeuronCores, batch-sharded).

Math: per SSM channel p, the complex diagonal recurrence
    x_t = lam_p * x_{t-1} + bu_t,   lam_p = r_p * exp(i*th_p)
is factored through the gauge x_t = exp(i*th_p*t) * y_t so the hardware
tensor_tensor_scan (DVE) runs with a REAL per-channel coefficient on the
re/im planes independently.

v2 layout decisions (from HW microbenchmarks):
  - All rotation tables, scan operands, and matmul operands are bf16:
    DVE tensor_tensor gets the 2x perf mode only when every operand is a
    2-byte dtype in SBUF; the scan runs at ~2.09 ns/col in bf16 vs ~3.9
    in f32; bf16 matmuls are 2x faster than f32r below peak p-state.
  - The scan coefficient must be fp32-accurate (a 2^-9 error in r is
    amplified by the ~1/(1-r) memory of the recurrence).  Split
    r = r_hi * r_lo with r_hi exactly representable in bf16 and
    |ln r_lo| <= 2^-9: the scan uses r_hi (exact in bf16), and the
    residual magnitudes r_lo^{-t} / r_lo^{+t} are folded into the
    pre/post rotation tables (bounded by e^4 over t<2048).
  - Rotation tables span the full L=2048 (no chunk phase folding), so B
    and C need only a single weight copy and the scan is one op per
    (half, comp) over the whole sequence: no carry chaining at all.
  - Act engine does every PSUM->SBUF copy (with the f32->bf16 downcast).
    GpSimd cannot touch PSUM and measured ~2.5 ns/col on bf16 adds, and
    running it concurrently with DVE regressed DVE throughput (SBUF port
    contention), so every rotation op stays on DVE.

Per core: 4 batches of the 32.
  - input tiles [l,h] -> PE transpose -> inT [h, l] (bf16)
  - B-proj:  bu[p_half, comp, t] = bw^T @ inT      (PE, bf16)
  - pre-rotation (packed complex mul)              (DVE bf16 2x)
  - scan y = r_hi*y + c along t, full L=2048       (DVE scan, bf16)
  - post-rotation -> x                             (DVE bf16 2x)
  - C-proj + D-residual -> out[t, h]               (PE, PSUM-accumulated)
"""

import numpy as np

BATCH, LENGTH, HIDDEN, P = 32, 2048, 128, 256
N_CORES = 8
BPC = BATCH // N_CORES          # batches per core
CH = 512                        # chunk size (PSUM tile granularity)
NCH = LENGTH // CH              # 4 chunks
NBLK = CH // 128                # 4 token-blocks of 128 per chunk

_COMPILED = {}


def _build_program():
    import concourse.bacc as bacc
    import concourse.mybir as mybir
    from concourse.ap import AP
    from concourse.tile import TileContext

    def dup2(a):
        """Insert a stride-0 'which' dim after the partition dim."""
        return AP(a.tensor, a.offset, [list(a.ap[0]), [0, 2],
                                       *[list(x) for x in a.ap[1:]]])

    f32 = mybir.dt.float32
    bf16 = mybir.dt.bfloat16

    nc = bacc.Bacc("TRN2", target_bir_lowering=False, debug=False,
                   num_devices=N_CORES)

    # ---- DRAM tensors (per-core) ----
    xin = nc.dram_tensor("xin", [BPC, LENGTH, HIDDEN], f32,
                         kind="ExternalInput").ap()
    # B weights [h, comp, half, p] / C weights [p, comp, half, h] (bf16)
    bw = nc.dram_tensor("bw", [HIDDEN, 2, 2, 128], bf16,
                        kind="ExternalInput").ap()
    cw = nc.dram_tensor("cw", [128, 2, 2, HIDDEN], bf16,
                        kind="ExternalInput").ap()
    # rotation tables [p, half, which, comp, t] over full L (bf16)
    tp = nc.dram_tensor("tp", [128, 2, 2, 2, LENGTH], bf16,
                        kind="ExternalInput").ap()
    tq = nc.dram_tensor("tq", [128, 2, 2, 2, LENGTH], bf16,
                        kind="ExternalInput").ap()
    # scan coefficient r_hi broadcast [p, half, t] (bf16-exact)
    rbc = nc.dram_tensor("rbc", [128, 2, LENGTH], bf16,
                         kind="ExternalInput").ap()
    dw = nc.dram_tensor("dw", [HIDDEN, HIDDEN], bf16,
                        kind="ExternalInput").ap()
    eye16 = nc.dram_tensor("eye16", [128, 128], bf16,
                           kind="ExternalInput").ap()
    out = nc.dram_tensor("out", [BPC, LENGTH, HIDDEN], f32,
                         kind="ExternalOutput").ap()

    with TileContext(nc) as tc:
        with (
            tc.tile_pool(name="const", bufs=1) as cpool,
            tc.tile_pool(name="inat", bufs=2) as inat_pool,
            tc.tile_pool(name="intp", bufs=2) as intr_pool,   # inT per batch
            tc.tile_pool(name="busb", bufs=2) as busb_pool,   # bu bf16 chunk
            tc.tile_pool(name="t12", bufs=2) as t12_pool,     # mul scratch
            tc.tile_pool(name="cbuf", bufs=2) as cbuf_pool,   # scan in (full L)
            tc.tile_pool(name="ybuf", bufs=1) as ybuf_pool,   # scan out (full L)
            tc.tile_pool(name="t34", bufs=2) as t34_pool,     # post scratch
            tc.tile_pool(name="otb", bufs=2) as otb_pool,     # outT staging
            tc.tile_pool(name="obuf", bufs=2) as obuf_pool,   # out staging
            tc.tile_pool(name="pst", bufs=2, space="PSUM") as pst,
            tc.tile_pool(name="psb", bufs=2, space="PSUM") as psb,
            tc.tile_pool(name="pso", bufs=1, space="PSUM") as pso,
            tc.tile_pool(name="psq", bufs=1, space="PSUM") as psq,
        ):
            # ---- constants to SBUF ----
            bw_t = cpool.tile([HIDDEN, 2, 2, 128], bf16, tag="bw")
            cw_t = cpool.tile([128, 2, 2, HIDDEN], bf16, tag="cw")
            tp_t = cpool.tile([128, 2, 2, 2, LENGTH], bf16, tag="tp")
            tq_t = cpool.tile([128, 2, 2, 2, LENGTH], bf16, tag="tq")
            rbc_t = cpool.tile([128, 2, LENGTH], bf16, tag="rbc")
            dw_t = cpool.tile([HIDDEN, HIDDEN], bf16, tag="dw")
            eye16_t = cpool.tile([128, 128], bf16, tag="eye16")
            # order matters: first consumers first.  tq/cw/dw/eye16 are
            # deferred into the batch-0 body so their DMAs don't contend
            # with batch-0's table reads on the SBUF ports.
            nc.sync.dma_start(eye16_t[:], eye16[:])
            nc.sync.dma_start(bw_t[:], bw[:])
            # batch-0 input next (see loop below), then tp chunk-sliced so
            # the first pre-rotation only waits on its own 1MB slice.
            dma_order = [
                lambda: [nc.sync.dma_start(tp_t[:, h, :, :, CH * J:CH * (J + 1)],
                                           tp[:, h, :, :, CH * J:CH * (J + 1)])
                         for J in range(NCH) for h in range(2)],
                lambda: nc.sync.dma_start(rbc_t[:], rbc[:]),
            ]

            inputs = {}

            def emit_input(b):
                # load + cast + transpose input: inT [h, l] bf16
                inT = intr_pool.tile([HIDDEN, LENGTH], bf16, tag="inT",
                                     name=f"inT{b}")
                nat = inat_pool.tile([128, LENGTH // 128, HIDDEN], f32,
                                     tag="nat")
                nat16 = inat_pool.tile([128, LENGTH // 128, HIDDEN], bf16,
                                       tag="nat16")
                nc.sync.dma_start(
                    nat[:], xin[b].rearrange("(n p) h -> p n h", p=128))
                if b == 0:
                    for fn in dma_order:
                        fn()
                nc.scalar.copy(nat16[:], nat[:])
                for q in range(NCH):
                    tpp = pst.tile([HIDDEN, 4, 128], bf16, tag="tpp")
                    for j in range(4):
                        nc.tensor.transpose(
                            tpp[:, j, :], nat16[:, 4 * q + j, :], eye16_t[:])
                    nc.scalar.copy(
                        inT[:, CH * q:CH * (q + 1)],
                        tpp[:].rearrange("h f t -> h (f t)"))
                inputs[b] = inT

            emit_input(0)
            for b in range(BPC):
                inT = inputs[b]
                # ---- B-proj + pre-rotation, chunk by chunk -> cc full L ----
                cc = cbuf_pool.tile([128, 2, 2, LENGTH], bf16, tag="cc",
                                    name=f"cc{b}")
                for J in range(NCH):
                    tsl = slice(CH * J, CH * (J + 1))
                    for half in range(2):
                        bu = psb.tile([128, 2, CH], f32, tag="bu")
                        for comp in range(2):
                            nc.tensor.matmul(
                                bu[:, comp, :],
                                bw_t[:, comp, half],
                                inT[:, tsl],
                                start=True, stop=True)
                        busb = busb_pool.tile([128, 2, CH], bf16, tag="busb")
                        nc.scalar.copy(busb[:], bu[:])
                        t12 = t12_pool.tile([128, 2, 2, CH], bf16, tag="t12")
                        # one mul op: busb broadcast (stride-0) over `which`
                        nc.vector.tensor_mul(
                            t12[:], dup2(busb[:]), tp_t[:, half, :, :, tsl])
                        # cc[half, comp] = t12[w, 0] + t12[w, 1] -> packed add
                        nc.vector.tensor_add(
                            cc[:, half, :, tsl],
                            t12[:, :, 0, :], t12[:, :, 1, :])

                if b == 0:
                    # deferred constant DMAs: queued behind batch-0 input
                    # work so they don't contend with batch-0 table reads
                    nc.sync.dma_start(tq_t[:], tq[:])
                    nc.sync.dma_start(cw_t[:], cw[:])
                    nc.sync.dma_start(dw_t[:], dw[:])

                # ---- scans: y = r_hi * y + c, full L per (half, comp) ----
                yy = ybuf_pool.tile([128, 2, 2, LENGTH], bf16, tag="yy",
                                    name=f"yy{b}")
                for half in range(2):
                    for comp in range(2):
                        nc.vector.tensor_tensor_scan(
                            yy[:, half, comp, :],
                            rbc_t[:, half],
                            cc[:, half, comp, :],
                            0.0,
                            op0=mybir.AluOpType.mult,
                            op1=mybir.AluOpType.add)

                # next batch's input phase overlaps this batch's scan/post
                if b + 1 < BPC:
                    emit_input(b + 1)

                # ---- post-rotation + C-proj + D-residual, per chunk ----
                osb = obuf_pool.tile([128, LENGTH // 128, HIDDEN], f32,
                                     tag="osb")
                for J in range(NCH):
                    tsl = slice(CH * J, CH * (J + 1))
                    t34s = []
                    for half in range(2):
                        t34 = t34_pool.tile([128, 2, 2, CH], bf16, tag="t34",
                                            name=f"t34_{b}_{J}_{half}")
                        nc.vector.tensor_mul(
                            t34[:], dup2(yy[:, half, :, tsl]),
                            tq_t[:, half, :, :, tsl])
                        t34s.append(t34)

                    # the post-rotation add (sum over y-comp c) is folded
                    # into the C-proj contraction: same cw lhsT for both c
                    outT = pso.tile([HIDDEN, CH], f32, tag="outT")
                    first = True
                    for comp in range(2):
                        for half in range(2):
                            for c in range(2):
                                nc.tensor.matmul(
                                    outT[:],
                                    cw_t[:, comp, half],
                                    t34s[half][:, comp, c, :],
                                    start=first, stop=False)
                                first = False
                    nc.tensor.matmul(
                        outT[:], dw_t[:], inT[:, tsl],
                        start=False, stop=True)
                    oT = otb_pool.tile([HIDDEN, CH], bf16, tag="oT")
                    nc.scalar.copy(oT[:], outT[:])
                    tpo = psq.tile([128, 4, HIDDEN], bf16, tag="tpo")
                    for i in range(NBLK):
                        nc.tensor.transpose(
                            tpo[:, i, :], oT[:, 128 * i:128 * (i + 1)],
                            eye16_t[:])
                    nc.scalar.copy(
                        osb[:, 4 * J:4 * (J + 1), :],
                        tpo[:].rearrange("t f h -> t (f h)").rearrange(
                            "t (f h) -> t f h", h=HIDDEN))
                nc.gpsimd.dma_start(
                    out[b].rearrange("(n p) h -> p n h", p=128), osb[:])

    nc.compile()
    return nc


def _host_constants(A_diag, G_diag, steps, B, C, D):
    """Parameter projection + eigenvalues + full-length rotation tables."""
    import ml_dtypes

    A = A_diag.astype(np.float64)
    G = G_diag.astype(np.float64)
    st = steps.astype(np.float64)
    step = 1.0 / (1.0 + np.exp(-st))
    g = np.maximum(G, 0.0)
    denom = np.maximum(step * step, 1e-6)
    s = step * g
    base = np.sqrt(np.maximum(1.0 + s, 1e-6))
    a_low = (2.0 + s - 2.0 * base) / denom
    a_high = (2.0 + s + 2.0 * base) / denom
    a = a_low + np.maximum(A - a_low, 0.0) - np.maximum(A - a_high, 0.0)
    S = 1.0 / (1.0 + step * g)
    T = S + 1.0 - step * step * S * a
    imag = np.sqrt(np.maximum(S - 0.25 * T * T, 0.0))
    lam = 0.5 * T + 1j * imag                      # [P] complex128
    r = np.abs(lam)
    th = np.angle(lam)

    # r = r_hi * r_lo with r_hi bf16-exact; fold r_lo^{+-t} into the tables
    r_hi = np.asarray(r.astype(np.float32), dtype=ml_dtypes.bfloat16)
    r_hi64 = r_hi.astype(np.float64)
    log_rlo = np.log(r) - np.log(r_hi64)           # |.| <= ~2^-9

    t = np.arange(LENGTH, dtype=np.float64)
    bf = ml_dtypes.bfloat16

    tp = np.zeros((128, 2, 2, 2, LENGTH), np.float32)
    tq = np.zeros((128, 2, 2, 2, LENGTH), np.float32)
    rbc = np.zeros((128, 2, LENGTH), np.float32)
    for half in range(2):
        psl = slice(128 * half, 128 * (half + 1))
        ang = th[psl, None] * t[None, :]
        cos_m = np.cos(ang)
        sin_m = np.sin(ang)
        mag_pre = np.exp(-log_rlo[psl, None] * t[None, :])   # r_lo^{-t}
        mag_post = np.exp(+log_rlo[psl, None] * t[None, :])  # r_lo^{+t}
        # pre: c = exp(-i th t) * r_lo^{-t} * bu
        tp[:, half, 0, 0, :] = cos_m * mag_pre
        tp[:, half, 0, 1, :] = sin_m * mag_pre
        tp[:, half, 1, 0, :] = -sin_m * mag_pre
        tp[:, half, 1, 1, :] = cos_m * mag_pre
        # post: x = exp(+i th t) * r_lo^{+t} * y
        tq[:, half, 0, 0, :] = cos_m * mag_post
        tq[:, half, 0, 1, :] = -sin_m * mag_post
        tq[:, half, 1, 0, :] = sin_m * mag_post
        tq[:, half, 1, 1, :] = cos_m * mag_post
        rbc[:, half, :] = r_hi64[psl, None]

    Br = B[..., 0].astype(np.float64)
    Bi = B[..., 1].astype(np.float64)
    Cr = C[..., 0].astype(np.float64)
    Ci = C[..., 1].astype(np.float64)
    bw = np.zeros((HIDDEN, 2, 2, 128), np.float32)
    cw = np.zeros((128, 2, 2, HIDDEN), np.float32)
    for half in range(2):
        psl = slice(128 * half, 128 * (half + 1))
        bw[:, 0, half] = Br[psl].T                 # lhsT [h, p]
        bw[:, 1, half] = Bi[psl].T
        cw[:, 0, half] = Cr[:, psl].T              # out = Cr*xr - Ci*xi
        cw[:, 1, half] = -Ci[:, psl].T

    dwm = np.diag(D.astype(np.float64)).astype(np.float32)
    return dict(
        bw=bw.astype(bf), cw=cw.astype(bf),
        tp=tp.astype(bf), tq=tq.astype(bf),
        rbc=rbc.astype(bf), dw=dwm.astype(bf),
        eye16=np.eye(128, dtype=np.float32).astype(bf),
    )


def kernel(inputs, A_diag, G_diag, steps, B, C, D):
    from concourse import bass_utils

    inputs = np.asarray(inputs, np.float32)
    consts = _host_constants(np.asarray(A_diag), np.asarray(G_diag),
                             np.asarray(steps), np.asarray(B), np.asarray(C),
                             np.asarray(D))

    if "prog" not in _COMPILED:
        _COMPILED["prog"] = _build_program()
    nc = _COMPILED["prog"]

    in_maps = []
    for core in range(N_CORES):
        m = dict(consts)
        m["xin"] = np.ascontiguousarray(inputs[BPC * core: BPC * (core + 1)])
        in_maps.append(m)
    res = bass_utils.run_bass_kernel_spmd(nc, in_maps,
                                          core_ids=list(range(N_CORES)))
    out = np.concatenate([res.results[i]["out"] for i in range(N_CORES)],
                         axis=0)
    return out.astype(np.float32)
